# revision 1
# baseline (speedup 1.0000x reference)
# Trainium2 Bass kernel for nn_Attention_88313117540497.
#
# Reference computation (per batch b of 128):
#   v = x_b @ Wv                      (196, 384) @ (384, 512)
#   conv: each of the 512 channels' 14x14 image convolved with a 27x27
#         kernel qk at padding 13 -> same 14x14 output
#   y = conv_out @ Wo + bo            (196, 512) @ (512, 384)
#
# Key observations:
#  1. A 27x27 kernel on a 14x14 image with padding 13 covers every input
#     pixel for every output pixel, so the conv is exactly a dense linear
#     map over the 196 positions: out[p] = sum_u M[p, u] * img[u] with
#     M[(p,q),(u,v)] = qk[13+u-p, 13+v-q]. M is shared across all
#     batches and channels, so conv == matmul with a 196x196 matrix.
#  2. The whole module is then  y_b = M @ x_b @ Wv @ Wo + bo.  Folding
#     W = Wv @ Wo (384x384, computed once on device) removes the
#     INNER=512 dimension entirely: y_b = M @ (x_b @ W) + bo, which
#     halves the FLOPs.
#  3. Feeding x transposed (features major) makes both matmuls natural
#     for the PE (contraction dim on partitions for both operands, no
#     on-device transposes):
#        H_b = XT_b.T @ W      (lhsT = XT_b, rhs = W)   -> (196, 384)
#        Y_b = MT.T @ H_b      (lhsT = MT,   rhs = H_b) -> (196, 384)
#     with MT = M^T. All matmuls have free dim N = 384 >= 256, so
#     float32r runs at 1 cycle/row (4x faster than plain fp32).
#
# Sharding: data-parallel over batch, 16 batches per core, weights
# replicated. No collectives needed.
#
# DMA layout (from trace analysis): dma_start triggers cost ~0.65us
# each on the issuing sequencer and every completion semaphore pays a
# ~2us HBM write-receipt, so transfers are few and large. Reads spread
# across all 16 SDMA engines, but SBUF->HBM writes are pinned to 4
# queues (~115GB/s), with the SWDGE path adding independent write
# lanes. So: x is host-packed feature-major (12.5KB DRAM rows), loaded
# as one DMA per 4-batch group on the sync sequencer (pure prefetch
# stream, no data-dependent stalls); y is written in a PE-native
# k-major grouped layout (host-reassembled) with the 128-partition
# chunk streamed per 2 batches on HWDGE and the 68-partition chunk per
# 4-batch group on SWDGE; small constants ride SWDGE early. Tiny fp32
# const-AP matmuls warm the PE clock before the first data lands.

import numpy as np

import concourse.bass as bass
from concourse import bacc
import concourse.mybir as mybir
import concourse.tile as tile
from concourse.bass_utils import run_bass_kernel_spmd

N_CORES = 8
B = 128                 # total batch
BPC = B // N_CORES      # batches per core
DIM = 384
INNER = 512
NPOS = 196              # 14*14 positions
IMG = 14
KS = 27                 # conv kernel size

F32 = mybir.dt.float32
F32R = mybir.dt.float32r

TOK_CHUNKS = [(0, 128), (128, 68)]
DCH = DIM // 128        # 3 feature chunks (contraction of stage 1)
ICH = INNER // 128      # 4 inner chunks (contraction of the fold)
# progressive X-load groups: a small first group lands quickly so the
# PE stream is gapless from first data; later groups amortize triggers
XGROUPS = [(0, 2), (2, 6), (8, 8)]   # (start batch, size)
GXMAX = max(sz for _, sz in XGROUPS)
GY = 4                  # batches per Y-store group
NGY = BPC // GY

# float32r (= tfloat32) runs at full PE rate for free dim >= 256. The
# BIR verifier requires producers of f32r matmul operands to write
# pre-rounded TF32: DMA-fed operands are rounded on the host, on-chip
# producers (PSUM evictions) write float32r directly.
MM_DT = F32R


def build_program():
    nc = bacc.Bacc("TRN2", debug=False)

    # x, feature-major: [feature chunk, partition (feature%128), token]
    xt_d = nc.dram_tensor("xt", [DCH, 128, BPC * NPOS], MM_DT,
                          kind="ExternalInput")
    w_d = nc.dram_tensor("w", [DIM, DIM], MM_DT, kind="ExternalInput")
    mt_d = nc.dram_tensor("mt", [NPOS, NPOS], MM_DT, kind="ExternalInput")
    bias_d = nc.dram_tensor("bias", [128, DIM], MM_DT, kind="ExternalInput")
    # y, PE-native: [group, p-chunk k, partition, batch-in-group, e]
    y_d = nc.dram_tensor("y", [NGY, 2, 128, GY, DIM], F32,
                         kind="ExternalOutput")

    GTMAX = GXMAX * NPOS
    xgrp = {}            # batch -> (start, size) of its group
    for s0, sz in XGROUPS:
        for bb in range(s0, s0 + sz):
            xgrp[bb] = (s0, sz)

    with tile.TileContext(nc) as tc:
        with (
            tc.tile_pool(name="const", bufs=1) as const,
            tc.tile_pool(name="work", bufs=2) as work,
            tc.tile_pool(name="psum", bufs=2, space="PSUM") as psum,
        ):
            dges = [nc.sync, nc.scalar]

            # ---- small constants via SWDGE (keeps HWDGE queues free) ----
            bias_sb = const.tile([128, DIM], MM_DT)
            nc.gpsimd.dma_start(bias_sb[:, :], bias_d[:, :])
            mt_sb = const.tile([128, 2 * NPOS], MM_DT)
            for uc, (u0, usz) in enumerate(TOK_CHUNKS):
                nc.gpsimd.dma_start(
                    mt_sb[:usz, uc * NPOS:(uc + 1) * NPOS],
                    mt_d[u0:u0 + usz, :],
                )

            # ---- folded weight W = Wv @ Wo (host-precomputed) ----
            w_sb = const.tile([128, DCH * DIM], MM_DT)
            nc.scalar.dma_start(
                w_sb[:, :].rearrange("p (c e) -> p c e", c=DCH),
                w_d.rearrange("(c p) e -> p c e", p=128),
            )

            # ---- PE warm-up on framework const APs (ready right after
            # instruction load, no DMA dependency): tiny fp32 matmuls keep
            # the tensor engine busy so the clock is ramped when the
            # stage-1 stream begins ----
            warm_c = nc.const_aps.tensor(1.0, (128, 1))
            for wi in range(28):
                warm = psum.tile([128, DIM], F32, tag="y1", name=f"warm{wi}")
                nc.tensor.matmul(
                    warm[0:1, 0:1],
                    lhsT=warm_c,
                    rhs=warm_c,
                    start=True,
                    stop=True,
                )

            # ---- main loop ----
            xt_t = None
            y_t = None
            for b in range(BPC):
                gstart, gsize = xgrp[b]
                gt = gsize * NPOS
                if b == gstart:
                    xt_t = work.tile([128, DCH * GTMAX], MM_DT, tag="xt",
                                     bufs=3, name=f"xt{gstart}")
                    ts0, ts1 = gstart * NPOS, (gstart + gsize) * NPOS
                    if gstart == 0:
                        # first group per feature chunk so stage-1 can start
                        # on chunk 0 while chunks 1-2 are in flight
                        for c in range(DCH):
                            nc.sync.dma_start(
                                xt_t[:, c * gt:(c + 1) * gt],
                                xt_d[c, :, ts0:ts1],
                            )
                    else:
                        nc.sync.dma_start(
                            xt_t[:, 0:DCH * gt].rearrange(
                                "p (c t) -> p c t", c=DCH),
                            xt_d[:, :, ts0:ts1].rearrange("c p t -> p c t"),
                        )
                if b % GY == 0:
                    # [k-chunk, batch-in-group, e] per partition
                    y_t = work.tile([128, 2 * GY * DIM], F32, tag="y", bufs=3,
                                    name=f"y{b // GY}")

                tok0 = (b - gstart) * NPOS
                bi = b % GY

                # stage 1: H_b = XT_b.T @ W  (tokens on partitions)
                h_t = work.tile([128, 2 * DIM], MM_DT, tag="h", bufs=4,
                                name=f"h{b}")
                for t, (u0, usz) in enumerate(TOK_CHUNKS):
                    ph = psum.tile([128, DIM], F32, tag=f"h{t}", name=f"ph{t}_{b}")
                    for c in range(DCH):
                        o = c * gt + tok0 + u0
                        nc.tensor.matmul(
                            ph[:usz, :],
                            lhsT=xt_t[:, o:o + usz],
                            rhs=w_sb[:, c * DIM:(c + 1) * DIM],
                            start=(c == 0),
                            stop=(c == DCH - 1),
                        )
                    nc.scalar.copy(h_t[:usz, t * DIM:(t + 1) * DIM],
                                   ph[:usz, :])

                # stage 2: Y_b = MT.T @ H_b + bias
                for t2, (p0, psz) in enumerate(TOK_CHUNKS):
                    py = psum.tile([128, DIM], F32, tag=f"y{t2}", name=f"py{t2}_{b}")
                    for uc, (u0, usz) in enumerate(TOK_CHUNKS):
                        nc.tensor.matmul(
                            py[:psz, :],
                            lhsT=mt_sb[:usz, uc * NPOS + p0:uc * NPOS + p0 + psz],
                            rhs=h_t[:usz, uc * DIM:(uc + 1) * DIM],
                            start=(uc == 0),
                            stop=(uc == 1),
                        )
                    nc.vector.tensor_add(
                        y_t[:psz, t2 * GY * DIM + bi * DIM:
                            t2 * GY * DIM + (bi + 1) * DIM],
                        py[:psz, :],
                        bias_sb[:psz, :].bitcast(F32),
                    )

                g = b // GY
                # stream k0 out per 2 batches on the (4-queue) HWDGE write
                # path so it is busy as early as possible; k1 rides the
                # otherwise-idle SWDGE path. The last group flushes in the
                # finest grains so the final transfer (and its completion
                # receipt) is as small as possible.
                last_group = (gstart + gsize == BPC)
                if last_group and b >= BPC - 2:
                    bi2 = b % GY
                    nc.sync.dma_start(
                        y_d[g, 0, :, bi2:bi2 + 1],
                        y_t[:, bi2 * DIM:(bi2 + 1) * DIM])
                elif b % 2 == 1:
                    h2 = (b % GY) // 2
                    nc.sync.dma_start(
                        y_d[g, 0, :, 2 * h2:2 * h2 + 2],
                        y_t[:, 2 * h2 * DIM:(2 * h2 + 2) * DIM])
                if last_group and b % 2 == 1:
                    h2 = (b % GY) // 2
                    nc.gpsimd.dma_start(
                        y_d[g, 1, 0:68, 2 * h2:2 * h2 + 2],
                        y_t[:68, (GY + 2 * h2) * DIM:(GY + 2 * h2 + 2) * DIM])
                elif b % GY == GY - 1:
                    nc.gpsimd.dma_start(
                        y_d[g, 1, 0:68], y_t[:68, GY * DIM:2 * GY * DIM])

    nc.compile()
    return nc


_PROGRAM = None


def _get_program():
    global _PROGRAM
    if _PROGRAM is None:
        _PROGRAM = build_program()
    return _PROGRAM


def _round_tf32(a):
    # round-to-nearest to the 10-bit TF32 mantissa (dtype-format conversion
    # for the float32r DRAM tensors)
    b = (a.view(np.uint32) + np.uint32(0x1000)) & np.uint32(0xFFFFE000)
    return b.view(np.float32)


def _host_prep(x, Wv, qk, Wo, bo):
    x = np.asarray(x, dtype=np.float32)
    # per-core feature-major token stream: (cores, 3, 128, BPC*196)
    XTC = np.ascontiguousarray(
        x.reshape(N_CORES, BPC * NPOS, DIM).transpose(0, 2, 1)
    ).reshape(N_CORES, DCH, 128, BPC * NPOS)
    XTC = _round_tf32(XTC)
    # one-time weight prep: fold the two projections (fp32 matmul), then
    # round to TF32 for the float32r stage-1 weights
    W = _round_tf32(np.ascontiguousarray(
        np.asarray(Wv, np.float32) @ np.asarray(Wo, np.float32)))
    # MT[(u,v),(p,q)] = qk[13+u-p, 13+v-q]  (pure gather, no arithmetic)
    qk2 = np.asarray(qk, np.float32).reshape(KS, KS)
    idx = (KS // 2) + np.arange(IMG)[:, None] - np.arange(IMG)[None, :]
    MT = _round_tf32(np.ascontiguousarray(
        qk2[idx[:, None, :, None], idx[None, :, None, :]].reshape(NPOS, NPOS)
    ))
    bias = np.ascontiguousarray(
        np.broadcast_to(np.asarray(bo, np.float32), (128, DIM))
    )
    return XTC, W, MT, bias


def _unpack_core(y2):
    # y2: [NGY, 2, 128, GY, DIM] -> (BPC, NPOS, DIM)
    out = np.empty((BPC, NPOS, DIM), np.float32)
    top = y2[:, 0].transpose(0, 2, 1, 3)          # [NGY, GY, 128, DIM]
    bot = y2[:, 1, 0:68].transpose(0, 2, 1, 3)    # [NGY, GY, 68, DIM]
    out[:, 0:128, :] = top.reshape(BPC, 128, DIM)
    out[:, 128:NPOS, :] = bot.reshape(BPC, 68, DIM)
    return out


def _run(x, Wv, qk, Wo, bo, **spmd_kwargs):
    XTC, W, MT, bias = _host_prep(x, Wv, qk, Wo, bo)
    nc = _get_program()
    in_maps = [
        {"xt": XTC[c], "w": W, "mt": MT, "bias": bias}
        for c in range(N_CORES)
    ]
    res = run_bass_kernel_spmd(nc, in_maps, list(range(N_CORES)), **spmd_kwargs)
    y = np.concatenate(
        [_unpack_core(res.results[c]["y"]) for c in range(N_CORES)], axis=0)
    return y, res


def kernel(x, Wv, qk, Wo, bo):
    y, _ = _run(x, Wv, qk, Wo, bo)
    return y



# revision 2
# speedup vs baseline: 1.2831x; 1.2831x over previous
# Trainium2 Bass kernel for nn_Attention_88313117540497.
#
# Reference computation (per batch b of 128):
#   v = x_b @ Wv                      (196, 384) @ (384, 512)
#   conv: each of the 512 channels' 14x14 image convolved with a 27x27
#         kernel qk at padding 13 -> same 14x14 output
#   y = conv_out @ Wo + bo            (196, 512) @ (512, 384)
#
# Algebra:
#  1. The 27x27 kernel at padding 13 on a 14x14 image covers every input
#     pixel for every output pixel, so the conv is a dense 196x196 linear
#     map M shared by all batches/channels:  y_b = M @ x_b @ (Wv@Wo) + bo.
#     Folding W = Wv@Wo (384x384) removes INNER=512 entirely.
#  2. PE mapping (out = lhsT.T @ rhs; lhsT stationary costs its free dim
#     in LDWEIGHTS columns, rhs streams N cycles):
#       stage A:  G^T_b = lhsT(X_b).T @ MT      X_b in NATURAL token-major
#                 layout, MT = M^T as rhs (N=196). 3 d-chunks x 2 v-chunks
#                 = 6 matmuls, 1176 cycles/batch.
#       stage B:  Y^T_b = lhsT(W).T @ G^T_b     W shared stationary,
#                 3 e-chunks x 3 d-chunks = 9 matmuls N=196, full 128x128
#                 array occupancy: 1764 cycles/batch (FLOP-optimal).
#  3. All matmul operands bf16 (halves DMA, enables Fast Weight Load so
#     LDWEIGHTS ~53ns hides under the 82ns matmul stream; rel err ~3e-3
#     vs the 2e-2 gate). Output evicted as fp16, bias fused into the
#     eviction (vector tensor_tensor add / scalar activation bias).
#
# PSUM: per batch 4 banks: G01 [128,392] (d-chunks 0,1 packed), G2
# [128,196], Y01 [128,392] (e-chunks 0,1), Y2 [128,196]; bufs=2 -> 8.
# Evictions split scalar/vector so each stays under the PE stream.
#
# Sharding: data-parallel over batch, 16 batches/core, no collectives.

import numpy as np
import ml_dtypes

import concourse.bass as bass
from concourse import bacc
import concourse.mybir as mybir
import concourse.tile as tile
from concourse.bass_utils import run_bass_kernel_spmd

N_CORES = 8
B = 128
BPC = B // N_CORES      # 16 batches per core
DIM = 384
NPOS = 196
IMG = 14
KS = 27

F32 = mybir.dt.float32
BF16 = mybir.dt.bfloat16
FP16 = mybir.dt.float16
BF16_NP = ml_dtypes.bfloat16

DCH = 3                  # 128-chunks of DIM
VCHUNKS = [(0, 128), (128, 68)]   # token chunks (contraction of stage A)
YW = 3 * NPOS            # 588 fp16 per batch in the output tile
GY = 4                   # batches per Y-store group
NGY = BPC // GY
# progressive x-load groups: small first group lands quickly
XGROUPS = [(0, 2), (2, 6), (8, 8)]
GXMAX = max(sz for _, sz in XGROUPS)
XB = 2 * DIM             # bf16 cols per batch in the x tile (both chunks)


def build_program():
    nc = bacc.Bacc("TRN2", debug=False)

    # x, natural token-major: [batch, token, feature] bf16
    x_d = nc.dram_tensor("x", [BPC, NPOS, DIM], BF16, kind="ExternalInput")
    w_d = nc.dram_tensor("w", [DIM, DIM], BF16, kind="ExternalInput")
    # mtb: packed [128, 392 mt | 392 bias01 | 1 bias2] bf16
    mtb_d = nc.dram_tensor("mtb", [128, 2 * NPOS + 2 * NPOS + 1], BF16,
                           kind="ExternalInput")
    # y, e-major fp16: [group, partition, batch-in-group, e-chunk, u]
    y_d = nc.dram_tensor("y", [NGY, 128, GY * YW], FP16,
                         kind="ExternalOutput")

    xgrp = {}
    for s0, sz in XGROUPS:
        for bb in range(s0, s0 + sz):
            xgrp[bb] = (s0, sz)

    with tile.TileContext(nc) as tc:
        with (
            tc.tile_pool(name="const", bufs=1) as const,
            tc.tile_pool(name="work", bufs=2) as work,
            tc.tile_pool(name="psum", bufs=2, space="PSUM") as psum,
        ):
            # ---- constants ----
            # mt chunks + bias, one HWDGE trigger on scalar
            mtb_sb = const.tile([128, 2 * NPOS + 2 * NPOS + 1], BF16)
            nc.scalar.dma_start(mtb_sb[:, :], mtb_d[:, :])
            w_sb = const.tile([128, DCH * DIM], BF16)
            nc.scalar.dma_start(
                w_sb[:, :].rearrange("p (c e) -> p c e", c=DCH),
                w_d.rearrange("(c p) e -> p c e", p=128),
            )
            mt = mtb_sb[:, 0:2 * NPOS]            # [v, 2*196]
            bias01 = mtb_sb[:, 2 * NPOS:4 * NPOS]  # [128, 392]
            bias2 = mtb_sb[:, 4 * NPOS:4 * NPOS + 1]

            # ---- PE warm-up (HAM ramp) on const APs ----
            warm_c = nc.const_aps.tensor(1.0, (128, 1))
            for wi in range(16):
                warm = psum.tile([128, NPOS], F32, tag="g2", name=f"warm{wi}")
                nc.tensor.matmul(
                    warm[0:1, 0:1], lhsT=warm_c, rhs=warm_c,
                    start=True, stop=True,
                )

            # ---- main loop ----
            x_t = None
            y_t = None
            for b in range(BPC):
                gstart, gsize = xgrp[b]
                if b == gstart:
                    x_t = work.tile([128, GXMAX * XB], BF16, tag="xt",
                                    bufs=3, name=f"xt{gstart}")
                    xv = x_t[:, 0:gsize * XB].rearrange(
                        "p (b c d) -> p b c d", b=gsize, c=2)
                    nc.sync.dma_start(
                        xv[:, :, 0, :],
                        x_d[gstart:gstart + gsize, 0:128, :]
                        .rearrange("b p d -> p b d"),
                    )
                    nc.sync.dma_start(
                        xv[0:68, :, 1, :],
                        x_d[gstart:gstart + gsize, 128:NPOS, :]
                        .rearrange("b p d -> p b d"),
                    )
                if b % GY == 0:
                    y_t = work.tile([128, GY * YW], FP16, tag="y", bufs=2,
                                    name=f"y{b // GY}")
                bi = b % GY
                xo = (b - gstart) * XB

                # stage A: G^T_b = X_b.T-contracted with MT (tokens = K)
                g01 = psum.tile([128, 2 * NPOS], F32, tag="g01",
                                name=f"g01_{b}")
                g2 = psum.tile([128, NPOS], F32, tag="g2", name=f"g2_{b}")
                for m in range(DCH):
                    dst = (g01[:, m * NPOS:(m + 1) * NPOS] if m < 2
                           else g2[:, :])
                    for v, (v0, vsz) in enumerate(VCHUNKS):
                        nc.tensor.matmul(
                            dst,
                            lhsT=x_t[0:vsz,
                                     xo + v * DIM + m * 128:
                                     xo + v * DIM + m * 128 + 128],
                            rhs=mt[0:vsz, v * NPOS:(v + 1) * NPOS],
                            start=(v == 0),
                            stop=(v == 1),
                        )

                # evict G^T to SBUF as bf16 (stage-B rhs)
                gt = work.tile([128, DCH * NPOS], BF16, tag="gt", bufs=3,
                               name=f"gt{b}")
                nc.scalar.copy(gt[:, 0:2 * NPOS], g01[:, :])
                nc.vector.tensor_copy(gt[:, 2 * NPOS:3 * NPOS], g2[:, :])

                # stage B: Y^T_b = W.T-contracted with G^T (d = K)
                y01 = psum.tile([128, 2 * NPOS], F32, tag="y01",
                                name=f"y01_{b}")
                y2 = psum.tile([128, NPOS], F32, tag="y2", name=f"y2_{b}")
                for e in range(DCH):
                    dst = (y01[:, e * NPOS:(e + 1) * NPOS] if e < 2
                           else y2[:, :])
                    for d in range(DCH):
                        nc.tensor.matmul(
                            dst,
                            lhsT=w_sb[:, d * DIM + e * 128:
                                      d * DIM + e * 128 + 128],
                            rhs=gt[:, d * NPOS:(d + 1) * NPOS],
                            start=(d == 0),
                            stop=(d == DCH - 1),
                        )

                # evict Y^T + bias to the fp16 output tile
                nc.vector.tensor_add(
                    y_t[:, bi * YW:bi * YW + 2 * NPOS],
                    y01[:, :], bias01,
                )
                nc.scalar.add(
                    y_t[:, bi * YW + 2 * NPOS:(bi + 1) * YW],
                    y2[:, :], bias2,
                )

                if bi == GY - 1:
                    g = b // GY
                    nc.gpsimd.dma_start(y_d[g], y_t[:, :])

    nc.compile()
    return nc


_PROGRAM = None


def _get_program():
    global _PROGRAM
    if _PROGRAM is None:
        _PROGRAM = build_program()
    return _PROGRAM


def _host_prep(x, Wv, qk, Wo, bo):
    x = np.asarray(x, dtype=np.float32)
    XC = x.reshape(N_CORES, BPC, NPOS, DIM).astype(BF16_NP)
    # fold the two projections once in fp32, round to bf16
    W = (np.asarray(Wv, np.float32) @ np.asarray(Wo, np.float32)).astype(BF16_NP)
    # MT[(u,v),(p,q)] = qk[13+u-p, 13+v-q]: conv as a 196x196 matmul
    qk2 = np.asarray(qk, np.float32).reshape(KS, KS)
    idx = (KS // 2) + np.arange(IMG)[:, None] - np.arange(IMG)[None, :]
    MT = np.ascontiguousarray(
        qk2[idx[:, None, :, None], idx[None, :, None, :]].reshape(NPOS, NPOS)
    ).astype(BF16_NP)
    bo = np.asarray(bo, np.float32)
    mtb = np.zeros((128, 4 * NPOS + 1), dtype=BF16_NP)
    mtb[:, 0:NPOS] = MT[0:128, :]
    mtb[0:68, NPOS:2 * NPOS] = MT[128:NPOS, :]
    p = np.arange(128)
    mtb[:, 2 * NPOS:3 * NPOS] = bo[p, None].astype(BF16_NP)       # e = p
    mtb[:, 3 * NPOS:4 * NPOS] = bo[128 + p, None].astype(BF16_NP)  # e = 128+p
    mtb[:, 4 * NPOS] = bo[256 + p].astype(BF16_NP)                 # e = 256+p
    return XC, W, mtb


def _unpack_core(y2):
    # y2: [NGY, 128, GY*588] fp16 -> (BPC, NPOS, DIM) f32
    a = np.asarray(y2, np.float32).reshape(NGY, 128, GY, DCH, NPOS)
    # out[b, u, e=128c+p] = a[g, p, bi, c, u]
    return np.ascontiguousarray(
        a.transpose(0, 2, 4, 3, 1).reshape(BPC, NPOS, DIM))


def _run(x, Wv, qk, Wo, bo, **spmd_kwargs):
    XC, W, mtb = _host_prep(x, Wv, qk, Wo, bo)
    nc = _get_program()
    in_maps = [
        {"x": XC[c], "w": W, "mtb": mtb}
        for c in range(N_CORES)
    ]
    res = run_bass_kernel_spmd(nc, in_maps, list(range(N_CORES)), **spmd_kwargs)
    y = np.concatenate(
        [_unpack_core(res.results[c]["y"]) for c in range(N_CORES)], axis=0)
    return y, res


def kernel(x, Wv, qk, Wo, bo):
    y, _ = _run(x, Wv, qk, Wo, bo)
    return y


# revision 3
# speedup vs baseline: 1.4282x; 1.1130x over previous
# Trainium2 Bass kernel for nn_Attention_88313117540497.
#
# Reference computation (per batch b of 128):
#   v = x_b @ Wv; conv2d of each channel's 14x14 image with 27x27 qk at
#   padding 13; y = conv_out @ Wo + bo.
#
# Algebra:
#  1. The padded 27x27 conv on 14x14 covers every pixel pair, so it is a
#     dense 196x196 map M shared across batches/channels:
#         y_b = M @ x_b @ (Wv@Wo) + bo,   W = Wv@Wo (384x384).
#  2. PE mapping (out = lhsT.T @ rhs; lhsT stationary, rhs streams N
#     cycles):
#       stage A:  G^T_b = lhsT(X_b).T @ MT    X_b in natural token-major
#                 layout, MT = M^T. 3 d-chunks x 2 v-chunks, N=196.
#       stage B:  Y^T_b = lhsT(W).T @ G^T_b   3 e-chunks x 3 d-chunks,
#                 N=196, full 128x128 array occupancy (FLOP-optimal).
#     All operands bf16: halves DMA, enables Fast Weight Load so the
#     LDWEIGHTS stream hides under the matmul stream.
#  3. Software pipeline: stage A of batch b+1 is issued before stage B of
#     batch b so the PE never stalls on the G eviction, and the PE stream
#     is gapless -> HAM clock-gate reaches 8/8 and stays there. A burst
#     of N=512 warm-up matmuls on const APs ramps HAM during the first
#     x DMA.
#  4. PSUM tiles use a 256-element column stride (chunks at 0/256/512)
#     so all three 196-wide chunks of G (or Y) sit in 2 banks without a
#     matmul output crossing a bank, and the eviction is a single
#     strided-AP op: scalar ACT copy for G (fp32->bf16), vector
#     tensor_tensor add for Y (bias fused, fp32->fp16). One op per
#     engine per batch.
#
# Sharding: data-parallel over batch, 16 batches/core, no collectives.

import numpy as np
import ml_dtypes

import concourse.bass as bass
from concourse import bacc
import concourse.mybir as mybir
import concourse.tile as tile
from concourse.bass_utils import run_bass_kernel_spmd

N_CORES = 8
B = 128
BPC = B // N_CORES      # 16 batches per core
DIM = 384
NPOS = 196
IMG = 14
KS = 27

F32 = mybir.dt.float32
BF16 = mybir.dt.bfloat16
FP16 = mybir.dt.float16
BF16_NP = ml_dtypes.bfloat16

DCH = 3                             # 128-chunks of DIM
VCHUNKS = [(0, 128), (128, 68)]     # token chunks (stage-A contraction)
YW = 3 * NPOS                       # 588 fp16 per batch in the output
PSTRIDE = 256                       # psum column stride between chunks
# progressive x-load groups: a small first group lands quickly
XGROUPS = [(0, 1), (1, 3), (4, 6), (10, 6)]
GXMAX = max(sz for _, sz in XGROUPS)
XB = 2 * DIM                        # bf16 cols per batch in the x tile
# y stores: taper the tail so the final transfer + receipt is small
YSTORES = [(0, 4), (4, 4), (8, 4), (12, 2), (14, 1), (15, 1)]
NWARM = 8


def build_program():
    nc = bacc.Bacc("TRN2", debug=False)

    # x, partition-major: [token, batch, feature] bf16 per core
    x_d = nc.dram_tensor("x", [NPOS, BPC, DIM], BF16, kind="ExternalInput")
    w_d = nc.dram_tensor("w", [DIM, DIM], BF16, kind="ExternalInput")
    # mtb: packed [128, 392 mt-chunks | 588 bias] bf16
    mtb_d = nc.dram_tensor("mtb", [128, 2 * NPOS + YW], BF16,
                           kind="ExternalInput")
    # y, e-major fp16: [partition, batch * (e-chunk, u)]
    y_d = nc.dram_tensor("y", [128, BPC * YW], FP16, kind="ExternalOutput")

    xgrp = {}
    for s0, sz in XGROUPS:
        for bb in range(s0, s0 + sz):
            xgrp[bb] = (s0, sz)

    with tile.TileContext(nc) as tc:
        with (
            tc.tile_pool(name="const", bufs=1) as const,
            tc.tile_pool(name="work", bufs=2) as work,
            tc.tile_pool(name="psum", bufs=2, space="PSUM") as psum,
        ):
            # ---- constants (scalar HWDGE; scalar is idle at start) ----
            mtb_sb = const.tile([128, 2 * NPOS + YW], BF16)
            nc.scalar.dma_start(mtb_sb[:, :], mtb_d[:, :])
            w_sb = const.tile([128, DCH * DIM], BF16)
            nc.scalar.dma_start(
                w_sb[:, :].rearrange("p (c e) -> p c e", c=DCH),
                w_d.rearrange("(c p) e -> p c e", p=128),
            )
            mt = mtb_sb[:, 0:2 * NPOS]
            bias = mtb_sb[:, 2 * NPOS:2 * NPOS + YW].rearrange(
                "p (c u) -> p c u", c=DCH)

            # ---- PE warm-up: dense N=512 matmuls ramp the HAM clock
            # gate to 8/8 while the first x group is still in flight ----
            warm_c = nc.const_aps.tensor(1.0, (128, 512))
            for wi in range(NWARM):
                warm = psum.tile([128, DCH * PSTRIDE], F32, tag="g",
                                 name=f"warm{wi}")
                nc.tensor.matmul(
                    warm[0:1, 0:512], lhsT=warm_c[:, 0:1], rhs=warm_c,
                    start=True, stop=True,
                )

            # ---- software-pipelined main loop: A(b) then B(b-1) ----
            x_t = None
            y_t = None
            gts = {}
            for b in range(BPC + 1):
                if b < BPC:
                    gstart, gsize = xgrp[b]
                    if b == gstart:
                        x_t = work.tile([128, GXMAX * XB], BF16, tag="xt",
                                        bufs=3, name=f"xt{gstart}")
                        xv = x_t[:, 0:gsize * XB].rearrange(
                            "p (b c d) -> p b c d", b=gsize, c=2)
                        nc.sync.dma_start(
                            xv[:, :, 0, :],
                            x_d[0:128, gstart:gstart + gsize, :])
                        nc.sync.dma_start(
                            xv[0:68, :, 1, :],
                            x_d[128:NPOS, gstart:gstart + gsize, :])
                    xo = (b - gstart) * XB

                    # stage A: G^T_b (d on partitions), tokens = K
                    gp = psum.tile([128, DCH * PSTRIDE], F32, tag="g",
                                   name=f"g{b}")
                    for m in range(DCH):
                        for v, (v0, vsz) in enumerate(VCHUNKS):
                            nc.tensor.matmul(
                                gp[:, m * PSTRIDE:m * PSTRIDE + NPOS],
                                lhsT=x_t[0:vsz,
                                         xo + v * DIM + m * 128:
                                         xo + v * DIM + m * 128 + 128],
                                rhs=mt[0:vsz, v * NPOS:(v + 1) * NPOS],
                                start=(v == 0),
                                stop=(v == 1),
                            )
                    # single strided eviction fp32->bf16 on scalar
                    gt = work.tile([128, DCH * NPOS], BF16, tag="gt",
                                   bufs=3, name=f"gt{b}")
                    nc.scalar.copy(
                        gt[:, :].rearrange("p (c u) -> p c u", c=DCH),
                        gp[:, :].rearrange("p (c s) -> p c s",
                                           c=DCH)[:, :, 0:NPOS],
                    )
                    gts[b] = gt

                if b >= 1:
                    bb = b - 1        # stage B batch
                    bi = bb % 4
                    if bi == 0:
                        y_t = work.tile([128, 4 * YW], FP16, tag="y",
                                        bufs=2, name=f"y{bb // 4}")
                    gt = gts.pop(bb)

                    # stage B: Y^T_b (e on partitions), d = K, W shared
                    yp = psum.tile([128, DCH * PSTRIDE], F32, tag="yp",
                                   name=f"yp{bb}")
                    for e in range(DCH):
                        for d in range(DCH):
                            nc.tensor.matmul(
                                yp[:, e * PSTRIDE:e * PSTRIDE + NPOS],
                                lhsT=w_sb[:, d * DIM + e * 128:
                                          d * DIM + e * 128 + 128],
                                rhs=gt[:, d * NPOS:(d + 1) * NPOS],
                                start=(d == 0),
                                stop=(d == DCH - 1),
                            )
                    # single strided eviction + bias, fp32->fp16, vector
                    nc.vector.tensor_add(
                        y_t[:, bi * YW:(bi + 1) * YW].rearrange(
                            "p (c u) -> p c u", c=DCH),
                        yp[:, :].rearrange("p (c s) -> p c s",
                                           c=DCH)[:, :, 0:NPOS],
                        bias,
                    )
                    for s0, ssz in YSTORES:
                        if bb == s0 + ssz - 1:
                            nc.gpsimd.dma_start(
                                y_d[:, s0 * YW:(s0 + ssz) * YW],
                                y_t[:, (s0 % 4) * YW:
                                    (s0 % 4 + ssz) * YW])

    nc.compile()
    return nc


_PROGRAM = None


def _get_program():
    global _PROGRAM
    if _PROGRAM is None:
        _PROGRAM = build_program()
    return _PROGRAM


def _host_prep(x, Wv, qk, Wo, bo):
    x = np.asarray(x, dtype=np.float32)
    # per-core partition-major: [core, token, batch, feature] bf16
    XC = np.ascontiguousarray(
        x.reshape(N_CORES, BPC, NPOS, DIM).transpose(0, 2, 1, 3)
    ).astype(BF16_NP)
    W = (np.asarray(Wv, np.float32) @ np.asarray(Wo, np.float32)).astype(BF16_NP)
    # MT[(u,v),(p,q)] = qk[13+u-p, 13+v-q]: conv as a 196x196 matmul
    qk2 = np.asarray(qk, np.float32).reshape(KS, KS)
    idx = (KS // 2) + np.arange(IMG)[:, None] - np.arange(IMG)[None, :]
    MT = np.ascontiguousarray(
        qk2[idx[:, None, :, None], idx[None, :, None, :]].reshape(NPOS, NPOS)
    ).astype(BF16_NP)
    bo = np.asarray(bo, np.float32)
    mtb = np.zeros((128, 2 * NPOS + YW), dtype=BF16_NP)
    mtb[:, 0:NPOS] = MT[0:128, :]
    mtb[0:68, NPOS:2 * NPOS] = MT[128:NPOS, :]
    be = bo.reshape(DCH, 128).astype(BF16_NP)    # bias[c][p] = bo[128c+p]
    for c in range(DCH):
        mtb[:, 2 * NPOS + c * NPOS:2 * NPOS + (c + 1) * NPOS] = be[c][:, None]
    return XC, W, mtb


def _unpack_core(y2):
    # y2: [128, BPC*588] fp16 -> (BPC, NPOS, DIM) f32
    a = np.asarray(y2, np.float32).reshape(128, BPC, DCH, NPOS)
    # out[b, u, e=128c+p] = a[p, b, c, u]
    return np.ascontiguousarray(
        a.transpose(1, 3, 2, 0).reshape(BPC, NPOS, DIM))


def _run(x, Wv, qk, Wo, bo, **spmd_kwargs):
    XC, W, mtb = _host_prep(x, Wv, qk, Wo, bo)
    nc = _get_program()
    in_maps = [
        {"x": XC[c], "w": W, "mtb": mtb}
        for c in range(N_CORES)
    ]
    res = run_bass_kernel_spmd(nc, in_maps, list(range(N_CORES)), **spmd_kwargs)
    y = np.concatenate(
        [_unpack_core(res.results[c]["y"]) for c in range(N_CORES)], axis=0)
    return y, res


def kernel(x, Wv, qk, Wo, bo):
    y, _ = _run(x, Wv, qk, Wo, bo)
    return y
